# revision 1
# baseline (speedup 1.0000x reference)
"""Trainium2 Bass kernel for a 2-layer LSTM encoder returning final (h, c).

Problem: enc_inp [B=128, T=1024, F=64]; two stacked LSTM layers with H=128.
Layer 2's initial state is layer 1's final state, so the 2048 recurrence
steps are strictly sequential — the kernel is latency-bound on the
h_{t-1} -> z_t -> gates -> c_t -> h_t dependency cycle.

Strategy (per NeuronCore, 8 cores data-parallel over batch, 16 samples each):
 - Layout: hidden/gate dim on partitions, batch on free dim. Per-step gate
   pre-activations in PSUM as [128, 4 gates * 16 batch].
 - Input contributions (W @ x_t) are computed by chunked GEMMs into PSUM
   ahead of the recurrence (one small GEMM piece woven into each step for a
   uniform PE cadence); the per-step U @ h matmuls accumulate on top (bf16).
 - All four gates use ONE sigmoid ACT per step; tanh(g) via
   tanh(x) = 2*sigmoid(2x)-1 with g-gate weights pre-scaled by 2 (host),
   gate column order (i, f, o, g2).
 - Cell update via fused scalar_tensor_tensor DVE ops; tanh(c) is the only
   other ACT. h is produced in bf16 (feeds next matmul), c stays fp32.
 - CRITICAL perf ingredient: on this hardware an engine that enters a
   blocked semaphore wait pays a multi-microsecond wake-up penalty. All
   engines on the dependency cycle are therefore padded with dummy
   (dependency-free) work sized so they arrive at each real instruction
   after its inputs are ready and never block.
"""

import numpy as np
import ml_dtypes

import concourse.bacc as bacc
import concourse.tile as tile
import concourse.mybir as mybir
from concourse.bass_utils import run_bass_kernel_spmd

N_CORES = 8
B, T_FULL, F, H = 128, 1024, 64, 128
BS = B // N_CORES  # batch per core
G4 = 4 * H
CHUNK = 32  # recurrence steps per PSUM chunk

BF16 = ml_dtypes.bfloat16

# Column permutation: keras gate order (i, f, g, o) -> (i, f, o, g)
_PERM = np.concatenate(
    [np.arange(0, H), np.arange(H, 2 * H), np.arange(3 * H, 4 * H),
     np.arange(2 * H, 3 * H)]
)

_ALU = mybir.AluOpType
_ACT = mybir.ActivationFunctionType

# Pad configuration: dummy-work sizes keeping each engine streaming.
# (n_ops, free_width) per pad site.
PADS = {
    "act_a": (1, 64),   # on ACT after sigma_t
    "act_b": (1, 64),   # on ACT after tanh_t
    "pe": (1, 256),     # dummy matmul rows after step's real matmuls
    "dve_a": (2, 64),   # dummy DVE ops between c-update and h_t (fills
                        # the tanh window so DVE doesn't block on th)
    "dve": (1, 64),     # dummy DVE ops after h_t
}


def _build(T, has_b1, reps=1, pads=None):
    """Build the SPMD Bass program for a T-step 2-layer LSTM."""
    pads = dict(PADS, **(pads or {}))
    bf = mybir.dt.bfloat16
    f32 = mybir.dt.float32

    nc = bacc.Bacc("TRN2", target_bir_lowering=False, debug=False,
                   enable_asserts=True, num_devices=N_CORES)

    xT = nc.dram_tensor("xT", [F + 1, T * BS], bf, kind="ExternalInput").ap()
    w0 = nc.dram_tensor("w0", [F + 1, G4], bf, kind="ExternalInput").ap()
    u0 = nc.dram_tensor("u0", [H, G4], bf, kind="ExternalInput").ap()
    w1 = nc.dram_tensor("w1", [H, G4], bf, kind="ExternalInput").ap()
    u1 = nc.dram_tensor("u1", [H, G4], bf, kind="ExternalInput").ap()
    if has_b1:
        b1 = nc.dram_tensor("b1", [1, G4], bf, kind="ExternalInput").ap()
    hc = nc.dram_tensor("hc", [H, 2 * BS], f32, kind="ExternalOutput").ap()

    assert T % CHUNK == 0
    PZW = 4 * CHUNK * BS  # psum tile width (f32 elements)

    with tile.TileContext(nc) as tc:
        with (
            tc.tile_pool(name="big", bufs=1) as big,
            tc.tile_pool(name="wts", bufs=1) as wts,
            tc.tile_pool(name="state", bufs=1) as state,
            tc.tile_pool(name="gates", bufs=4) as gates,
            tc.tile_pool(name="tmps", bufs=4) as tmps,
            tc.tile_pool(name="hsmall", bufs=4) as hsmall,
            tc.tile_pool(name="pz", bufs=2, space="PSUM") as pzpool,
        ):
            # --- load inputs ---
            xTs = big.tile([F + 1, T * BS], bf, tag="xT")
            nc.sync.dma_start(out=xTs, in_=xT)
            hs0 = big.tile([H, T * BS], bf, tag="hs0")

            w0s = wts.tile([F + 1, G4], bf, tag="w0")
            u0s = wts.tile([H, G4], bf, tag="u0")
            w1s = wts.tile([H, G4], bf, tag="w1")
            u1s = wts.tile([H, G4], bf, tag="u1")
            nc.sync.dma_start(out=w0s, in_=w0)
            nc.sync.dma_start(out=u0s, in_=u0)
            nc.sync.dma_start(out=w1s, in_=w1)
            nc.sync.dma_start(out=u1s, in_=u1)
            b1s = None
            ones = None
            if has_b1:
                b1s = wts.tile([1, G4], bf, tag="b1")
                nc.sync.dma_start(out=b1s, in_=b1)
                ones = state.tile([1, BS], bf, tag="ones")
                nc.vector.memset(ones, 1.0)

            c = state.tile([H, BS], f32, tag="c")
            h0 = state.tile([H, BS], bf, tag="h0")
            hc_stage = state.tile([H, 2 * BS], f32, tag="hc_stage")
            # dedicated dependency-free tiles for pad (dummy) ops
            da_in = state.tile([H, 64], f32, tag="da_in")
            da_out = state.tile([H, 64], f32, tag="da_out")
            dv_in = state.tile([H, 64], f32, tag="dv_in")
            dv_out = state.tile([H, 64], f32, tag="dv_out")
            nc.vector.memset(da_in, 0.0)
            nc.vector.memset(dv_in, 0.0)

            n_act_a, w_act_a = pads["act_a"]
            n_act_b, w_act_b = pads["act_b"]
            n_pe, w_pe = pads["pe"]
            n_dve_a, w_dve_a = pads["dve_a"]
            n_dve, w_dve = pads["dve"]

            def pad_act(n, w):
                for _ in range(n):
                    nc.scalar.activation(da_out[:, :w], da_in[:, :w],
                                         _ACT.Sigmoid)

            def pad_pe(n, w, pzn, k):
                # dummy matmuls into the NEXT chunk's psum tile, targeting
                # a bank whose first xz piece (start=True, whole-bank
                # reset) has not executed yet: the reset wipes the dummy
                # garbage. Bank j's reset runs at k==8j, so at step k we
                # can safely write bank k//8 + 1. No cross-engine deps ->
                # PE never blocks on these.
                if pzn is None or k >= 24:
                    return
                j = k // 8 + 1
                pzn3 = pzn.rearrange("p (g n) -> p g n", g=4)
                for _ in range(n):
                    nc.tensor.matmul(pzn3[:, j, 0:w], w0s[:, 0:H],
                                     xTs[:, 0:w], start=False, stop=False,
                                     skip_group_check=True)

            def pad_dve(n, w):
                for _ in range(n):
                    nc.vector.tensor_scalar_mul(dv_out[:, :w], dv_in[:, :w],
                                                1.0)

            NPIECE = CHUNK  # one GEMM piece per step of the chunk

            def emit_gemm_piece(pz, w_s, x_s, c0, piece):
                """One xz GEMM piece: steps [c0,c0+CHUNK), gate piece//8,
                step-eighth piece%8. Each gate owns a full PSUM bank; only
                the first piece of a bank may use start=True (start resets
                the whole bank)."""
                j, q = piece // 8, piece % 8
                pz3 = pz.rearrange("p (g n) -> p g n", g=4)
                qw = CHUNK * BS // 8
                cols = slice(c0 * BS + q * qw, c0 * BS + (q + 1) * qw)
                nc.tensor.matmul(
                    pz3[:, j, q * qw:(q + 1) * qw],
                    w_s[:, j * H:(j + 1) * H],
                    x_s[:, cols],
                    start=(q == 0), stop=False, skip_group_check=True,
                )

            def emit_layer(layer, x_s, w_s, u_s, b_s, h_prev, last_layer):
                pz_cur = pzpool.tile([H, PZW], f32, tag="pz")
                for p in range(NPIECE):
                    emit_gemm_piece(pz_cur, w_s, x_s, 0, p)
                pz_next = None
                for t in range(T):
                    k = t % CHUNK
                    if k == 0 and t > 0:
                        pz_cur = pz_next
                    pz3 = pz_cur.rearrange("p (g n) -> p g n", g=4)
                    sl = slice(k * BS, (k + 1) * BS)
                    # one GEMM piece per step for the next chunk (uniform
                    # PE cadence); allocate the next psum tile at k==0
                    if t + CHUNK < T:
                        if k == 0:
                            pz_next = pzpool.tile([H, PZW], f32, tag="pz")
                        emit_gemm_piece(pz_next, w_s, x_s,
                                        (t // CHUNK + 1) * CHUNK, k)
                    for j in range(4):
                        nc.tensor.matmul(
                            pz3[:, j, sl],
                            u_s[:, j * H:(j + 1) * H],
                            h_prev,
                            start=False, stop=b_s is None,
                            skip_group_check=True,
                        )
                        if b_s is not None:
                            nc.tensor.matmul(
                                pz3[:, j, sl],
                                b_s[:, j * H:(j + 1) * H],
                                ones,
                                start=False, stop=True, skip_group_check=True,
                            )
                    pad_pe(n_pe, w_pe,
                           pz_next if t + CHUNK < T else None, k)

                    S = gates.tile([H, 4 * BS], f32, tag="S")
                    S3 = S.rearrange("p (g n) -> p g n", g=4)
                    nc.scalar.activation(S3, pz3[:, :, sl], _ACT.Sigmoid)
                    pad_act(n_act_a, w_act_a)
                    # S columns: [sig(i) | sig(f) | sig(o) | sig(2 zg)]
                    si = S[:, 0:BS]
                    sf = S[:, BS:2 * BS]
                    so = S[:, 2 * BS:3 * BS]
                    sg = S[:, 3 * BS:4 * BS]
                    ig2 = tmps.tile([H, BS], f32, tag="ig2")
                    # ig2 = (sig(2zg) - 0.5) * i  ==  i * tanh(zg) / 2
                    nc.vector.scalar_tensor_tensor(
                        ig2, sg, 0.5, si, _ALU.subtract, _ALU.mult)
                    fc = tmps.tile([H, BS], f32, tag="fc")
                    nc.vector.tensor_mul(fc, c, sf)
                    # c = 2*ig2 + fc
                    nc.vector.scalar_tensor_tensor(
                        c, ig2, 2.0, fc, _ALU.mult, _ALU.add)
                    pad_dve(n_dve_a, w_dve_a)
                    th = tmps.tile([H, BS], f32, tag="th")
                    nc.scalar.activation(th, c, _ACT.Tanh)
                    pad_act(n_act_b, w_act_b)
                    last_step = last_layer and t == T - 1
                    if last_step:
                        nc.vector.tensor_mul(hc_stage[:, 0:BS], th, so)
                    else:
                        if layer == 0:
                            h_prev = hs0[:, t * BS:(t + 1) * BS]
                        else:
                            h_prev = hsmall.tile([H, BS], bf, tag="h1")
                        nc.vector.tensor_mul(h_prev, th, so)
                    pad_dve(n_dve, w_dve)
                return h_prev

            def body():
                nc.vector.memset(c, 0.0)
                nc.vector.memset(h0, 0.0)
                hlast0 = emit_layer(0, xTs, w0s, u0s, None, h0,
                                    last_layer=False)
                emit_layer(1, hs0, w1s, u1s, b1s, hlast0, last_layer=True)
                nc.vector.tensor_copy(hc_stage[:, BS:2 * BS], c)
                nc.sync.dma_start(out=hc, in_=hc_stage)

            if reps == 1:
                body()
            else:
                with tc.For_i(0, reps, 1):
                    body()

    nc.finalize()
    return nc


_CACHE = {}


def _get_program(T, has_b1, reps=1, pads=None):
    key = (T, has_b1, reps, tuple(sorted((pads or {}).items())))
    if key not in _CACHE:
        _CACHE[key] = _build(T, has_b1, reps, pads)
    return _CACHE[key]


def _prep_weights(W0, U0, b0, W1, U1, b1):
    """Permute gates to (i,f,o,g), scale g-block by 2, cast bf16."""
    def prep(M):
        Mp = np.asarray(M, np.float32)[..., _PERM].copy()
        Mp[..., 3 * H:4 * H] *= 2.0
        return Mp
    w0a = np.concatenate([prep(W0), prep(b0)[None, :]], axis=0).astype(BF16)
    u0a = prep(U0).astype(BF16)
    w1a = prep(W1).astype(BF16)
    u1a = prep(U1).astype(BF16)
    b1p = prep(b1)[None, :].astype(BF16)
    has_b1 = bool(np.any(np.asarray(b1) != 0))
    return w0a, u0a, w1a, u1a, b1p, has_b1


def _prep_x(enc_inp, T):
    """Per-core transposed+augmented inputs: [F+1, T*BS] bf16."""
    outs = []
    for k in range(N_CORES):
        xk = np.asarray(enc_inp[k * BS:(k + 1) * BS, :T], np.float32)
        xk = np.ascontiguousarray(xk.transpose(2, 1, 0)).reshape(F, T * BS)
        xa = np.concatenate([xk, np.ones((1, T * BS), np.float32)], axis=0)
        outs.append(xa.astype(BF16))
    return outs


def run_lstm(enc_inp, W0, U0, b0, W1, U1, b1, T=T_FULL, reps=1, pads=None):
    w0a, u0a, w1a, u1a, b1p, has_b1 = _prep_weights(W0, U0, b0, W1, U1, b1)
    xs = _prep_x(enc_inp, T)
    nc = _get_program(T, has_b1, reps, pads)
    in_maps = []
    for k in range(N_CORES):
        m = {"xT": xs[k], "w0": w0a, "u0": u0a, "w1": w1a, "u1": u1a}
        if has_b1:
            m["b1"] = b1p
        in_maps.append(m)
    res = run_bass_kernel_spmd(nc, in_maps, list(range(N_CORES)))
    h = np.empty((B, H), np.float32)
    c = np.empty((B, H), np.float32)
    for k in range(N_CORES):
        hck = res.results[k]["hc"]  # [H, 2*BS]
        h[k * BS:(k + 1) * BS] = hck[:, :BS].T
        c[k * BS:(k + 1) * BS] = hck[:, BS:].T
    return h, c


def kernel(enc_inp, W0, U0, b0, W1, U1, b1):
    h, c = run_lstm(np.asarray(enc_inp), np.asarray(W0), np.asarray(U0),
                    np.asarray(b0), np.asarray(W1), np.asarray(U1),
                    np.asarray(b1), T=T_FULL)
    return h, c



# revision 4
# speedup vs baseline: 13.7687x; 13.7687x over previous
"""Trainium2 Bass kernel for a 2-layer LSTM encoder returning final (h, c).

Problem: enc_inp [B=128, T=1024, F=64]; two stacked LSTM layers with H=128.
Layer 2's initial state is layer 1's final state, so the 2048 recurrence
steps are strictly sequential — the kernel is latency-bound on the
h_{t-1} -> z_t -> gates -> c_t -> h_t dependency cycle.

Strategy (per NeuronCore, 8 cores data-parallel over batch, 16 samples each):
 - Layout: hidden/gate dim on partitions, batch on free dim. Per-step gate
   pre-activations in PSUM as [128, 4 gates * 16 batch].
 - Input contributions (W @ x_t) are computed by chunked GEMMs into PSUM
   ahead of the recurrence (one small GEMM piece woven into each step for a
   uniform PE cadence); the per-step U @ h matmuls accumulate on top (bf16).
 - All four gates use ONE sigmoid ACT per step; tanh(g) via
   tanh(x) = 2*sigmoid(2x)-1 with g-gate weights pre-scaled by 2 (host),
   gate column order (i, f, o, g2).
 - Cell update via fused scalar_tensor_tensor DVE ops; tanh(c) is the only
   other ACT. h is produced in bf16 (feeds next matmul), c stays fp32.
 - CRITICAL perf ingredient: on this hardware an engine that enters a
   blocked (unsatisfied) semaphore wait pays a multi-microsecond wake-up
   penalty. Every engine on the dependency cycle therefore runs a fully
   saturated per-step instruction queue: real ops plus dependency-free
   filler ops sized so all per-engine queues sum to a common cycle length
   L* that is >= the cross-engine dependency chain. In steady state each
   real op's inputs are ready before the engine reaches it, so no engine
   ever blocks. PE filler matmuls write a dedicated PSUM scratch bank
   (CHUNK=16 keeps the xz double-buffer small enough to free that bank).
"""

import numpy as np
import ml_dtypes

import concourse.bacc as bacc
import concourse.tile as tile
import concourse.mybir as mybir
from concourse.bass_utils import run_bass_kernel_spmd

N_CORES = 8
B, T_FULL, F, H = 128, 1024, 64, 128
BS = B // N_CORES  # batch per core
G4 = 4 * H
CHUNK = 16  # recurrence steps per PSUM chunk
QW = 64     # moving columns per xz GEMM piece

BF16 = ml_dtypes.bfloat16

# Column permutation: keras gate order (i, f, g, o) -> (i, f, o, g)
_PERM = np.concatenate(
    [np.arange(0, H), np.arange(H, 2 * H), np.arange(3 * H, 4 * H),
     np.arange(2 * H, 3 * H)]
)

_ALU = mybir.AluOpType
_ACT = mybir.ActivationFunctionType

# Pad configuration: (n_ops, free_width) per pad site. Sized so each
# engine's per-step queue sums to ~the same cycle length L* (model est):
#   ACT: sig(238) + padA + tanh(198) + padB          = L*
#   DVE: ig2+fc+c (231) + padDa + ho(77) + padD      = L*
#   PE:  xz piece(80) + 4 U mm(240) + n pad mm       = L*
# and each pad covers the window until its successor's input is ready.
PADS = {
    "act_a": (1, 245),   # between sigmoid and tanh
    "act_b": (1, 349),   # after tanh until next step's sigmoid
    "pe": (5, 512),      # dummy matmuls into PSUM scratch after U mms
    "dve_a": (1, 454),   # between c-update and ho
    "dve": (2, 549),     # after ho until next step's ig2
}
PAD_W = 1024  # width of pad tiles


def _build(T, has_b1, reps=1, pads=None):
    """Build the SPMD Bass program for a T-step 2-layer LSTM."""
    pads = dict(PADS, **(pads or {}))
    bf = mybir.dt.bfloat16
    f32 = mybir.dt.float32

    nc = bacc.Bacc("TRN2", target_bir_lowering=False, debug=False,
                   enable_asserts=True, num_devices=N_CORES)

    xT = nc.dram_tensor("xT", [F + 1, T * BS], bf, kind="ExternalInput").ap()
    w0 = nc.dram_tensor("w0", [F + 1, G4], bf, kind="ExternalInput").ap()
    u0 = nc.dram_tensor("u0", [H, G4], bf, kind="ExternalInput").ap()
    w1 = nc.dram_tensor("w1", [H, G4], bf, kind="ExternalInput").ap()
    u1 = nc.dram_tensor("u1", [H, G4], bf, kind="ExternalInput").ap()
    if has_b1:
        b1 = nc.dram_tensor("b1", [1, G4], bf, kind="ExternalInput").ap()
    hc = nc.dram_tensor("hc", [H, 2 * BS], f32, kind="ExternalOutput").ap()

    assert T % CHUNK == 0
    PZW = 4 * CHUNK * BS           # psum tile width (f32 elements)
    NPIECE = CHUNK                 # xz GEMM pieces per chunk (1 per step)
    PPG = CHUNK * BS // QW         # pieces per gate
    assert PPG * 4 == NPIECE

    with tile.TileContext(nc) as tc:
        with (
            tc.tile_pool(name="big", bufs=1) as big,
            tc.tile_pool(name="wts", bufs=1) as wts,
            tc.tile_pool(name="state", bufs=1) as state,
            tc.tile_pool(name="gates", bufs=4) as gates,
            tc.tile_pool(name="tmps", bufs=4) as tmps,
            tc.tile_pool(name="hsmall", bufs=4) as hsmall,
            tc.tile_pool(name="pz", bufs=2, space="PSUM") as pzpool,
            tc.tile_pool(name="pescr", bufs=1, space="PSUM") as pescr,
        ):
            # --- load inputs ---
            xTs = big.tile([F + 1, T * BS], bf, tag="xT")
            nc.sync.dma_start(out=xTs, in_=xT)
            hs0 = big.tile([H, T * BS], bf, tag="hs0")

            w0s = wts.tile([F + 1, G4], bf, tag="w0")
            u0s = wts.tile([H, G4], bf, tag="u0")
            w1s = wts.tile([H, G4], bf, tag="w1")
            u1s = wts.tile([H, G4], bf, tag="u1")
            nc.sync.dma_start(out=w0s, in_=w0)
            nc.sync.dma_start(out=u0s, in_=u0)
            nc.sync.dma_start(out=w1s, in_=w1)
            nc.sync.dma_start(out=u1s, in_=u1)
            b1s = None
            ones = None
            if has_b1:
                b1s = wts.tile([1, G4], bf, tag="b1")
                nc.sync.dma_start(out=b1s, in_=b1)
                ones = state.tile([1, BS], bf, tag="ones")
                nc.vector.memset(ones, 1.0)

            c = state.tile([H, BS], f32, tag="c")
            h0 = state.tile([H, BS], bf, tag="h0")
            hc_stage = state.tile([H, 2 * BS], f32, tag="hc_stage")
            # dedicated dependency-free tiles for pad (dummy) ops
            da_in = state.tile([H, PAD_W], f32, tag="da_in")
            da_out = state.tile([H, PAD_W], f32, tag="da_out")
            dv_in = state.tile([H, PAD_W], f32, tag="dv_in")
            dv_out = state.tile([H, PAD_W], f32, tag="dv_out")
            nc.vector.memset(da_in, 0.0)
            nc.vector.memset(dv_in, 0.0)
            # PE dummy-matmul scratch: one PSUM bank, never read
            scr = pescr.tile([H, 512], f32, tag="scr")

            n_act_a, w_act_a = pads["act_a"]
            n_act_b, w_act_b = pads["act_b"]
            n_pe, w_pe = pads["pe"]
            n_dve_a, w_dve_a = pads["dve_a"]
            n_dve, w_dve = pads["dve"]
            assert w_pe <= 512 and w_act_a <= PAD_W and w_act_b <= PAD_W
            assert w_dve_a <= PAD_W and w_dve <= PAD_W

            def pad_act(n, w):
                for _ in range(n):
                    nc.scalar.activation(da_out[:, :w], da_in[:, :w],
                                         _ACT.Sigmoid)

            def pad_pe(n, w):
                # dependency-free dummy matmuls into the scratch bank;
                # start=True wipes it each time so garbage never overflows.
                for _ in range(n):
                    nc.tensor.matmul(scr[:, 0:w], u0s[:, 0:H], u0s[:, 0:w],
                                     start=True, stop=True,
                                     skip_group_check=True)

            def pad_dve(n, w):
                for _ in range(n):
                    nc.vector.tensor_scalar_mul(dv_out[:, :w], dv_in[:, :w],
                                                1.0)

            def emit_gemm_piece(pz, w_s, x_s, c0, piece):
                """One xz GEMM piece for the chunk starting at step c0.
                piece = j*PPG + q: gate j, column-eighth q. start=True only
                on the first piece of each PSUM bank (2 gates per bank;
                start resets the whole bank)."""
                j, q = piece // PPG, piece % PPG
                pz3 = pz.rearrange("p (g n) -> p g n", g=4)
                cols = slice(c0 * BS + q * QW, c0 * BS + (q + 1) * QW)
                nc.tensor.matmul(
                    pz3[:, j, q * QW:(q + 1) * QW],
                    w_s[:, j * H:(j + 1) * H],
                    x_s[:, cols],
                    start=(q == 0 and j % 2 == 0), stop=False,
                    skip_group_check=True,
                )

            def emit_layer(layer, x_s, w_s, u_s, b_s, h_prev, last_layer):
                pz_cur = pzpool.tile([H, PZW], f32, tag="pz")
                for p in range(NPIECE):
                    emit_gemm_piece(pz_cur, w_s, x_s, 0, p)
                pz_next = None
                for t in range(T):
                    k = t % CHUNK
                    if k == 0 and t > 0:
                        pz_cur = pz_next
                    pz3 = pz_cur.rearrange("p (g n) -> p g n", g=4)
                    sl = slice(k * BS, (k + 1) * BS)
                    # one GEMM piece per step for the next chunk (uniform
                    # PE cadence); allocate the next psum tile at k==0
                    if t + CHUNK < T:
                        if k == 0:
                            pz_next = pzpool.tile([H, PZW], f32, tag="pz")
                        emit_gemm_piece(pz_next, w_s, x_s,
                                        (t // CHUNK + 1) * CHUNK, k)
                    for j in range(4):
                        nc.tensor.matmul(
                            pz3[:, j, sl],
                            u_s[:, j * H:(j + 1) * H],
                            h_prev,
                            start=False, stop=b_s is None,
                            skip_group_check=True,
                        )
                        if b_s is not None:
                            nc.tensor.matmul(
                                pz3[:, j, sl],
                                b_s[:, j * H:(j + 1) * H],
                                ones,
                                start=False, stop=True, skip_group_check=True,
                            )
                    pad_pe(n_pe, w_pe)

                    S = gates.tile([H, 4 * BS], f32, tag="S")
                    S3 = S.rearrange("p (g n) -> p g n", g=4)
                    nc.scalar.activation(S3, pz3[:, :, sl], _ACT.Sigmoid)
                    pad_act(n_act_a, w_act_a)
                    # S columns: [sig(i) | sig(f) | sig(o) | sig(2 zg)]
                    si = S[:, 0:BS]
                    sf = S[:, BS:2 * BS]
                    so = S[:, 2 * BS:3 * BS]
                    sg = S[:, 3 * BS:4 * BS]
                    ig2 = tmps.tile([H, BS], f32, tag="ig2")
                    # ig2 = (sig(2zg) - 0.5) * i  ==  i * tanh(zg) / 2
                    nc.vector.scalar_tensor_tensor(
                        ig2, sg, 0.5, si, _ALU.subtract, _ALU.mult)
                    fc = tmps.tile([H, BS], f32, tag="fc")
                    nc.vector.tensor_mul(fc, c, sf)
                    # c = 2*ig2 + fc
                    nc.vector.scalar_tensor_tensor(
                        c, ig2, 2.0, fc, _ALU.mult, _ALU.add)
                    pad_dve(n_dve_a, w_dve_a)
                    th = tmps.tile([H, BS], f32, tag="th")
                    nc.scalar.activation(th, c, _ACT.Tanh)
                    pad_act(n_act_b, w_act_b)
                    last_step = last_layer and t == T - 1
                    if last_step:
                        nc.vector.tensor_mul(hc_stage[:, 0:BS], th, so)
                    else:
                        if layer == 0:
                            h_prev = hs0[:, t * BS:(t + 1) * BS]
                        else:
                            h_prev = hsmall.tile([H, BS], bf, tag="h1")
                        nc.vector.tensor_mul(h_prev, th, so)
                    pad_dve(n_dve, w_dve)
                return h_prev

            def body():
                nc.vector.memset(c, 0.0)
                nc.vector.memset(h0, 0.0)
                hlast0 = emit_layer(0, xTs, w0s, u0s, None, h0,
                                    last_layer=False)
                emit_layer(1, hs0, w1s, u1s, b1s, hlast0, last_layer=True)
                nc.vector.tensor_copy(hc_stage[:, BS:2 * BS], c)
                nc.sync.dma_start(out=hc, in_=hc_stage)

            if reps == 1:
                body()
            else:
                with tc.For_i(0, reps, 1):
                    body()

    nc.finalize()
    return nc


_CACHE = {}


def _get_program(T, has_b1, reps=1, pads=None):
    key = (T, has_b1, reps, tuple(sorted((pads or {}).items())))
    if key not in _CACHE:
        _CACHE[key] = _build(T, has_b1, reps, pads)
    return _CACHE[key]


def _prep_weights(W0, U0, b0, W1, U1, b1):
    """Permute gates to (i,f,o,g), scale g-block by 2, cast bf16."""
    def prep(M):
        Mp = np.asarray(M, np.float32)[..., _PERM].copy()
        Mp[..., 3 * H:4 * H] *= 2.0
        return Mp
    w0a = np.concatenate([prep(W0), prep(b0)[None, :]], axis=0).astype(BF16)
    u0a = prep(U0).astype(BF16)
    w1a = prep(W1).astype(BF16)
    u1a = prep(U1).astype(BF16)
    b1p = prep(b1)[None, :].astype(BF16)
    has_b1 = bool(np.any(np.asarray(b1) != 0))
    return w0a, u0a, w1a, u1a, b1p, has_b1


def _prep_x(enc_inp, T):
    """Per-core transposed+augmented inputs: [F+1, T*BS] bf16."""
    outs = []
    for k in range(N_CORES):
        xk = np.asarray(enc_inp[k * BS:(k + 1) * BS, :T], np.float32)
        xk = np.ascontiguousarray(xk.transpose(2, 1, 0)).reshape(F, T * BS)
        xa = np.concatenate([xk, np.ones((1, T * BS), np.float32)], axis=0)
        outs.append(xa.astype(BF16))
    return outs


def run_lstm(enc_inp, W0, U0, b0, W1, U1, b1, T=T_FULL, reps=1, pads=None):
    w0a, u0a, w1a, u1a, b1p, has_b1 = _prep_weights(W0, U0, b0, W1, U1, b1)
    xs = _prep_x(enc_inp, T)
    nc = _get_program(T, has_b1, reps, pads)
    in_maps = []
    for k in range(N_CORES):
        m = {"xT": xs[k], "w0": w0a, "u0": u0a, "w1": w1a, "u1": u1a}
        if has_b1:
            m["b1"] = b1p
        in_maps.append(m)
    res = run_bass_kernel_spmd(nc, in_maps, list(range(N_CORES)))
    h = np.empty((B, H), np.float32)
    c = np.empty((B, H), np.float32)
    for k in range(N_CORES):
        hck = res.results[k]["hc"]  # [H, 2*BS]
        h[k * BS:(k + 1) * BS] = hck[:, :BS].T
        c[k * BS:(k + 1) * BS] = hck[:, BS:].T
    return h, c


def kernel(enc_inp, W0, U0, b0, W1, U1, b1):
    h, c = run_lstm(np.asarray(enc_inp), np.asarray(W0), np.asarray(U0),
                    np.asarray(b0), np.asarray(W1), np.asarray(U1),
                    np.asarray(b1), T=T_FULL)
    return h, c


# revision 5
# speedup vs baseline: 26.3361x; 1.9127x over previous
"""Trainium2 Bass kernel for a 2-layer LSTM encoder returning final (h, c).

Problem: enc_inp [B=128, T=1024, F=64]; two stacked LSTM layers with H=128.
Layer 2's initial state is layer 1's final state, so the 2048 recurrence
steps are strictly sequential — the kernel is latency-bound on the
h_{t-1} -> z_t -> gates -> c_t -> h_t dependency cycle.

Strategy (per NeuronCore, 8 cores data-parallel over batch, 16 samples each):
 - Layout: hidden/gate dim on partitions, batch on free dim. Per-step gate
   pre-activations in PSUM as [128, 4 gates * 16 batch].
 - Input contributions (W @ x_t) are computed by chunked GEMMs into PSUM
   ahead of the recurrence (one small GEMM piece woven into each step for a
   uniform PE cadence); the per-step U @ h matmuls accumulate on top (bf16).
 - All four gates use ONE sigmoid ACT per step; tanh(g) via
   tanh(x) = 2*sigmoid(2x)-1 with g-gate weights pre-scaled by 2 (host),
   gate column order (i, f, o, g2).
 - Cell update via fused scalar_tensor_tensor DVE ops; tanh(c) is the only
   other ACT. h is produced in bf16 (feeds next matmul), c stays fp32.
 - CRITICAL perf ingredient: on this hardware an engine that enters a
   blocked (unsatisfied) semaphore wait pays a multi-microsecond wake-up
   penalty. Every engine on the dependency cycle therefore runs a fully
   saturated per-step instruction queue: real ops plus dependency-free
   filler ops sized so all per-engine queues sum to a common cycle length
   L* that is >= the cross-engine dependency chain. In steady state each
   real op's inputs are ready before the engine reaches it, so no engine
   ever blocks. PE filler matmuls write a dedicated PSUM scratch bank
   (CHUNK=16 keeps the xz double-buffer small enough to free that bank).
"""

import numpy as np
import ml_dtypes

import concourse.bacc as bacc
import concourse.tile as tile
import concourse.mybir as mybir
from concourse.bass_utils import run_bass_kernel_spmd

N_CORES = 8
B, T_FULL, F, H = 128, 1024, 64, 128
BS = B // N_CORES  # batch per core
G4 = 4 * H
CHUNK = 16  # recurrence steps per PSUM chunk
QW = 64     # moving columns per xz GEMM piece

BF16 = ml_dtypes.bfloat16

# Column permutation: keras gate order (i, f, g, o) -> (i, f, o, g)
_PERM = np.concatenate(
    [np.arange(0, H), np.arange(H, 2 * H), np.arange(3 * H, 4 * H),
     np.arange(2 * H, 3 * H)]
)

_ALU = mybir.AluOpType
_ACT = mybir.ActivationFunctionType

# Pad configuration: (n_ops, free_width) per pad site. Sized so each
# engine's per-step queue sums to ~the same cycle length L* (model est):
#   ACT: sig(238) + padA + tanh(198) + padB          = L*
#   DVE: ig2+fc+c (231) + padDa + ho(77) + padD      = L*
#   PE:  xz piece(80) + 4 U mm(240) + n pad mm       = L*
# and each pad covers the window until its successor's input is ready.
PADS = {
    "act_a": (1, 191),   # between sigmoid and tanh
    "act_b": (1, 283),   # after tanh until next step's sigmoid
    "pe": (4, 512),      # dummy matmuls into PSUM scratch after U mms
    "dve_a": (1, 396),   # between c-update and ho
    "dve": (2, 481),     # after ho until next step's ig2
}
PAD_W = 1024  # width of pad tiles


def _build(T, has_b1, reps=1, pads=None):
    """Build the SPMD Bass program for a T-step 2-layer LSTM."""
    pads = dict(PADS, **(pads or {}))
    bf = mybir.dt.bfloat16
    f32 = mybir.dt.float32

    nc = bacc.Bacc("TRN2", target_bir_lowering=False, debug=False,
                   enable_asserts=True, num_devices=N_CORES)

    xT = nc.dram_tensor("xT", [F + 1, T * BS], bf, kind="ExternalInput").ap()
    w0 = nc.dram_tensor("w0", [F + 1, G4], bf, kind="ExternalInput").ap()
    u0 = nc.dram_tensor("u0", [H, G4], bf, kind="ExternalInput").ap()
    w1 = nc.dram_tensor("w1", [H, G4], bf, kind="ExternalInput").ap()
    u1 = nc.dram_tensor("u1", [H, G4], bf, kind="ExternalInput").ap()
    if has_b1:
        b1 = nc.dram_tensor("b1", [1, G4], bf, kind="ExternalInput").ap()
    hc = nc.dram_tensor("hc", [H, 2 * BS], f32, kind="ExternalOutput").ap()

    assert T % CHUNK == 0
    PZW = 4 * CHUNK * BS           # psum tile width (f32 elements)
    NPIECE = CHUNK                 # xz GEMM pieces per chunk (1 per step)
    PPG = CHUNK * BS // QW         # pieces per gate
    assert PPG * 4 == NPIECE

    with tile.TileContext(nc) as tc:
        with (
            tc.tile_pool(name="big", bufs=1) as big,
            tc.tile_pool(name="wts", bufs=1) as wts,
            tc.tile_pool(name="state", bufs=1) as state,
            tc.tile_pool(name="gates", bufs=4) as gates,
            tc.tile_pool(name="tmps", bufs=4) as tmps,
            tc.tile_pool(name="hsmall", bufs=4) as hsmall,
            tc.tile_pool(name="pz", bufs=2, space="PSUM") as pzpool,
            tc.tile_pool(name="pescr", bufs=1, space="PSUM") as pescr,
        ):
            # --- load inputs ---
            xTs = big.tile([F + 1, T * BS], bf, tag="xT")
            nc.sync.dma_start(out=xTs, in_=xT)
            hs0 = big.tile([H, T * BS], bf, tag="hs0")

            w0s = wts.tile([F + 1, G4], bf, tag="w0")
            u0s = wts.tile([H, G4], bf, tag="u0")
            w1s = wts.tile([H, G4], bf, tag="w1")
            u1s = wts.tile([H, G4], bf, tag="u1")
            nc.sync.dma_start(out=w0s, in_=w0)
            nc.sync.dma_start(out=u0s, in_=u0)
            nc.sync.dma_start(out=w1s, in_=w1)
            nc.sync.dma_start(out=u1s, in_=u1)
            b1s = None
            ones = None
            if has_b1:
                b1s = wts.tile([1, G4], bf, tag="b1")
                nc.sync.dma_start(out=b1s, in_=b1)
                ones = state.tile([1, BS], bf, tag="ones")
                nc.vector.memset(ones, 1.0)

            c = state.tile([H, BS], f32, tag="c")
            h0 = state.tile([H, BS], bf, tag="h0")
            hc_stage = state.tile([H, 2 * BS], f32, tag="hc_stage")
            # dedicated dependency-free tiles for pad (dummy) ops
            da_in = state.tile([H, PAD_W], f32, tag="da_in")
            da_out = state.tile([H, PAD_W], f32, tag="da_out")
            dv_in = state.tile([H, PAD_W], f32, tag="dv_in")
            dv_out = state.tile([H, PAD_W], f32, tag="dv_out")
            nc.vector.memset(da_in, 0.0)
            nc.vector.memset(dv_in, 0.0)
            # PE dummy-matmul scratch: one PSUM bank, never read
            scr = pescr.tile([H, 512], f32, tag="scr")

            n_act_a, w_act_a = pads["act_a"]
            n_act_b, w_act_b = pads["act_b"]
            n_pe, w_pe = pads["pe"]
            n_dve_a, w_dve_a = pads["dve_a"]
            n_dve, w_dve = pads["dve"]
            assert w_pe <= 512 and w_act_a <= PAD_W and w_act_b <= PAD_W
            assert w_dve_a <= PAD_W and w_dve <= PAD_W

            def pad_act(n, w):
                for _ in range(n):
                    nc.scalar.activation(da_out[:, :w], da_in[:, :w],
                                         _ACT.Sigmoid)

            def pad_pe(n, w):
                # dependency-free dummy matmuls into the scratch bank;
                # start=True wipes it each time so garbage never overflows.
                for _ in range(n):
                    nc.tensor.matmul(scr[:, 0:w], u0s[:, 0:H], u0s[:, 0:w],
                                     start=True, stop=True,
                                     skip_group_check=True)

            def pad_dve(n, w):
                for _ in range(n):
                    nc.vector.tensor_scalar_mul(dv_out[:, :w], dv_in[:, :w],
                                                1.0)

            def emit_gemm_piece(pz, w_s, x_s, c0, piece):
                """One xz GEMM piece for the chunk starting at step c0.
                piece = j*PPG + q: gate j, column-eighth q. start=True only
                on the first piece of each PSUM bank (2 gates per bank;
                start resets the whole bank)."""
                j, q = piece // PPG, piece % PPG
                pz3 = pz.rearrange("p (g n) -> p g n", g=4)
                cols = slice(c0 * BS + q * QW, c0 * BS + (q + 1) * QW)
                nc.tensor.matmul(
                    pz3[:, j, q * QW:(q + 1) * QW],
                    w_s[:, j * H:(j + 1) * H],
                    x_s[:, cols],
                    start=(q == 0 and j % 2 == 0), stop=False,
                    skip_group_check=True,
                )

            def emit_layer(layer, x_s, w_s, u_s, b_s, h_prev, last_layer):
                pz_cur = pzpool.tile([H, PZW], f32, tag="pz")
                for p in range(NPIECE):
                    emit_gemm_piece(pz_cur, w_s, x_s, 0, p)
                pz_next = None
                for t in range(T):
                    k = t % CHUNK
                    if k == 0 and t > 0:
                        pz_cur = pz_next
                    pz3 = pz_cur.rearrange("p (g n) -> p g n", g=4)
                    sl = slice(k * BS, (k + 1) * BS)
                    # one GEMM piece per step for the next chunk (uniform
                    # PE cadence); allocate the next psum tile at k==0
                    if t + CHUNK < T:
                        if k == 0:
                            pz_next = pzpool.tile([H, PZW], f32, tag="pz")
                        emit_gemm_piece(pz_next, w_s, x_s,
                                        (t // CHUNK + 1) * CHUNK, k)
                    for j in range(4):
                        nc.tensor.matmul(
                            pz3[:, j, sl],
                            u_s[:, j * H:(j + 1) * H],
                            h_prev,
                            start=False, stop=b_s is None,
                            skip_group_check=True,
                        )
                        if b_s is not None:
                            nc.tensor.matmul(
                                pz3[:, j, sl],
                                b_s[:, j * H:(j + 1) * H],
                                ones,
                                start=False, stop=True, skip_group_check=True,
                            )
                    pad_pe(n_pe, w_pe)

                    S = gates.tile([H, 4 * BS], f32, tag="S")
                    S3 = S.rearrange("p (g n) -> p g n", g=4)
                    nc.scalar.activation(S3, pz3[:, :, sl], _ACT.Sigmoid)
                    pad_act(n_act_a, w_act_a)
                    # S columns: [sig(i) | sig(f) | sig(o) | sig(2 zg)]
                    si = S[:, 0:BS]
                    sf = S[:, BS:2 * BS]
                    so = S[:, 2 * BS:3 * BS]
                    sg = S[:, 3 * BS:4 * BS]
                    ig2 = tmps.tile([H, BS], f32, tag="ig2")
                    # ig2 = (sig(2zg) - 0.5) * i  ==  i * tanh(zg) / 2
                    nc.vector.scalar_tensor_tensor(
                        ig2, sg, 0.5, si, _ALU.subtract, _ALU.mult)
                    fc = tmps.tile([H, BS], f32, tag="fc")
                    nc.vector.tensor_mul(fc, c, sf)
                    # c = 2*ig2 + fc
                    nc.vector.scalar_tensor_tensor(
                        c, ig2, 2.0, fc, _ALU.mult, _ALU.add)
                    pad_dve(n_dve_a, w_dve_a)
                    th = tmps.tile([H, BS], f32, tag="th")
                    nc.scalar.activation(th, c, _ACT.Tanh)
                    pad_act(n_act_b, w_act_b)
                    last_step = last_layer and t == T - 1
                    if last_step:
                        nc.vector.tensor_mul(hc_stage[:, 0:BS], th, so)
                    else:
                        if layer == 0:
                            h_prev = hs0[:, t * BS:(t + 1) * BS]
                        else:
                            h_prev = hsmall.tile([H, BS], bf, tag="h1")
                        nc.vector.tensor_mul(h_prev, th, so)
                    pad_dve(n_dve, w_dve)
                return h_prev

            def body():
                nc.vector.memset(c, 0.0)
                nc.vector.memset(h0, 0.0)
                hlast0 = emit_layer(0, xTs, w0s, u0s, None, h0,
                                    last_layer=False)
                emit_layer(1, hs0, w1s, u1s, b1s, hlast0, last_layer=True)
                nc.vector.tensor_copy(hc_stage[:, BS:2 * BS], c)
                nc.sync.dma_start(out=hc, in_=hc_stage)

            if reps == 1:
                body()
            else:
                with tc.For_i(0, reps, 1):
                    body()

    nc.finalize()
    return nc


_CACHE = {}


def _get_program(T, has_b1, reps=1, pads=None):
    key = (T, has_b1, reps, tuple(sorted((pads or {}).items())))
    if key not in _CACHE:
        _CACHE[key] = _build(T, has_b1, reps, pads)
    return _CACHE[key]


def _prep_weights(W0, U0, b0, W1, U1, b1):
    """Permute gates to (i,f,o,g), scale g-block by 2, cast bf16."""
    def prep(M):
        Mp = np.asarray(M, np.float32)[..., _PERM].copy()
        Mp[..., 3 * H:4 * H] *= 2.0
        return Mp
    w0a = np.concatenate([prep(W0), prep(b0)[None, :]], axis=0).astype(BF16)
    u0a = prep(U0).astype(BF16)
    w1a = prep(W1).astype(BF16)
    u1a = prep(U1).astype(BF16)
    b1p = prep(b1)[None, :].astype(BF16)
    has_b1 = bool(np.any(np.asarray(b1) != 0))
    return w0a, u0a, w1a, u1a, b1p, has_b1


def _prep_x(enc_inp, T):
    """Per-core transposed+augmented inputs: [F+1, T*BS] bf16."""
    outs = []
    for k in range(N_CORES):
        xk = np.asarray(enc_inp[k * BS:(k + 1) * BS, :T], np.float32)
        xk = np.ascontiguousarray(xk.transpose(2, 1, 0)).reshape(F, T * BS)
        xa = np.concatenate([xk, np.ones((1, T * BS), np.float32)], axis=0)
        outs.append(xa.astype(BF16))
    return outs


def run_lstm(enc_inp, W0, U0, b0, W1, U1, b1, T=T_FULL, reps=1, pads=None):
    w0a, u0a, w1a, u1a, b1p, has_b1 = _prep_weights(W0, U0, b0, W1, U1, b1)
    xs = _prep_x(enc_inp, T)
    nc = _get_program(T, has_b1, reps, pads)
    in_maps = []
    for k in range(N_CORES):
        m = {"xT": xs[k], "w0": w0a, "u0": u0a, "w1": w1a, "u1": u1a}
        if has_b1:
            m["b1"] = b1p
        in_maps.append(m)
    res = run_bass_kernel_spmd(nc, in_maps, list(range(N_CORES)))
    h = np.empty((B, H), np.float32)
    c = np.empty((B, H), np.float32)
    for k in range(N_CORES):
        hck = res.results[k]["hc"]  # [H, 2*BS]
        h[k * BS:(k + 1) * BS] = hck[:, :BS].T
        c[k * BS:(k + 1) * BS] = hck[:, BS:].T
    return h, c


def kernel(enc_inp, W0, U0, b0, W1, U1, b1):
    h, c = run_lstm(np.asarray(enc_inp), np.asarray(W0), np.asarray(U0),
                    np.asarray(b0), np.asarray(W1), np.asarray(U1),
                    np.asarray(b1), T=T_FULL)
    return h, c
